# revision 25
# baseline (speedup 1.0000x reference)
"""Causal self-attention (B=2048, T=128, C=192, H=6, D=32) on 8 TRN2 cores.

Data-parallel over batch: 256 elems/core. v3: quad-batched qkv (N=512
matmuls), single fused exp per pair, broadcast-AP softmax normalize (one
tensor_tensor instead of 12 tensor_scalars), reduce/copies balanced across
scalar/vector/gpsimd engines.

Per quad (4 elems):
  x --DMA--> xf --cast--> x16 --PE transpose--> xT (+ones row)
  qT/kT = W^T @ xT (N=512, bias fused);  v = xT^T @ Wv (bias fused)
Per pair (2 elems):
  S_h[t,s] = q_h^T k_h (row-tiled PE, 4-concurrent)
  P = exp(S) one ACTIVATE;  Pm = P*tril (bcast mul);  rsum (DVE reduce);
  rrec duplicated-pair recip;  Pn = Pm*rrec_bcast (one mul, 2x mode);
  P^T via PE transpose;  y^T = V^T P^T (col-tiled);  out = y W_p -> HBM.
"""

import sys

sys.path.insert(0, "/opt/trn_rl_repo")

import numpy as np
import ml_dtypes

N_CORES = 8
B, T, C = 2048, 128, 192
NH, HD = 6, 32
BL = B // N_CORES  # 256 per core

_CACHE = {}


def _build(bl):
    from contextlib import ExitStack

    import concourse.bass as bass
    import concourse.mybir as mybir
    import concourse.tile as tile
    from concourse import bacc

    fp32 = mybir.dt.float32
    bf16 = mybir.dt.bfloat16
    AF = mybir.ActivationFunctionType

    nc = bacc.Bacc("TRN2", target_bir_lowering=False, debug=False)

    x_d = nc.dram_tensor("x", [bl, T, C], fp32, kind="ExternalInput")
    wA_d = nc.dram_tensor("wA", [128, 704], bf16, kind="ExternalInput")
    wB_d = nc.dram_tensor("wB", [65, 704], bf16, kind="ExternalInput")
    wpA_d = nc.dram_tensor("wpA", [128, 192], bf16, kind="ExternalInput")
    wpB_d = nc.dram_tensor("wpB", [65, 192], bf16, kind="ExternalInput")
    tril_d = nc.dram_tensor("trilR", [128, 12, 128], bf16, kind="ExternalInput")
    idr_d = nc.dram_tensor("identR", [128, 128], bf16, kind="ExternalInput")
    out_d = nc.dram_tensor("out", [bl, T, C], fp32, kind="ExternalOutput")

    with tile.TileContext(nc) as tc, ExitStack() as ctx:
        consts = ctx.enter_context(tc.tile_pool(name="consts", bufs=1))
        sbq = ctx.enter_context(tc.tile_pool(name="sbq", bufs=2))
        sbp = ctx.enter_context(tc.tile_pool(name="sbp", bufs=4))
        ps = ctx.enter_context(
            tc.tile_pool(name="ps", bufs=1, space=bass.MemorySpace.PSUM)
        )

        wA = consts.tile([128, 704], bf16)
        nc.sync.dma_start(wA[:], wA_d[:])
        wB = consts.tile([65, 704], bf16)
        nc.sync.dma_start(wB[:], wB_d[:])
        wpA = consts.tile([128, 192], bf16)
        nc.sync.dma_start(wpA[:], wpA_d[:])
        wpB = consts.tile([65, 192], bf16)
        nc.sync.dma_start(wpB[:], wpB_d[:])
        trilR = consts.tile([128, 12, 128], bf16)
        nc.sync.dma_start(trilR[:], tril_d[:])
        ident = consts.tile([128, 128], bf16)
        nc.sync.dma_start(ident[:], idr_d[:])

        # q/k packed as 3-head blocks: head h sits at rowgroup 32*(h%3) of
        # j-block h//3. S psum bank = h%3 = the PE row-group, so concurrent
        # row-tiled matmuls always hit distinct banks and same-bank matmuls
        # share a row-group (strictly serialized by the PE). S issue order
        # fills banks 0-1 first so exp1 can start early.
        # pidx: position in the packed [128, 12, 128] P16 layout.
        SORDER = [(0, 0), (0, 1), (0, 3), (0, 4), (1, 0), (1, 1), (1, 3),
                  (1, 4), (0, 2), (0, 5), (1, 2), (1, 5)]
        SMAP = {}
        fill = [0, 0, 0]
        base = [0, 4, 8]
        for ee, h in SORDER:
            b = h % 3
            SMAP[(ee, h)] = (b, fill[b], base[b] + fill[b])
            fill[b] += 1

        def pt(tag, shape, dtype=fp32, name=None):
            return ps.tile(shape, dtype, tag=tag, name=name or f"ps_{tag}")

        for q in range(bl // 4):
            # ---------------- quad phase: load, transpose, qkv ----------
            xf = sbq.tile([128, 4, 192], fp32, tag="xf")
            nc.sync.dma_start(
                xf[:], x_d[4 * q : 4 * q + 4].rearrange("e t c -> t e c")
            )
            x16 = sbq.tile([128, 4, 256], bf16, tag="x16")
            nc.vector.tensor_copy(x16[:, :, 0:192], xf[:])

            xTp = pt("xt1", [128, 4, 2, 128], bf16)
            for e in range(4):
                nc.tensor.transpose(xTp[:, e, 0, :], x16[:, e, 0:128], ident[:])
                nc.tensor.transpose(xTp[:, e, 1, :], x16[:, e, 128:256], ident[:])
            xT = sbq.tile([128, 4, 2, 128], bf16, tag="xT")
            nc.vector.tensor_copy(xT[:], xTp[:])
            nc.gpsimd.memset(xT[64:65, :, 1, :], 1.0)

            # qkT j-blocks [q h0-2 | q h3-5 | k h0-2 | k h3-5] (3 heads + pad
            # per block). j0-j2 -> 3-bank tag shared with S; j3 -> own bank.
            qkT = sbq.tile([128, 4, 4, 128], bf16, tag="qkT")
            T3a = pt("qs3", [128, 3, 4, 128])
            T3b = pt("qs1", [128, 1, 4, 128])
            for j in range(4):
                dst = T3a[:, j, :, :] if j < 3 else T3b[:, 0, :, :]
                nc.tensor.matmul(
                    dst,
                    wA[:, 128 * j : 128 * (j + 1)],
                    xT[:, :, 0, :],
                    start=True,
                    stop=False,
                )
                nc.tensor.matmul(
                    dst,
                    wB[:, 128 * j : 128 * (j + 1)],
                    xT[0:65, :, 1, :],
                    start=False,
                    stop=True,
                )
            nc.scalar.copy(qkT[:, 0:3, :, :], T3a[:])
            nc.scalar.copy(qkT[:, 3, :, :], T3b[:, 0, :, :])

            v16 = sbq.tile([128, 4, 192], bf16, tag="v16")
            vp = pt("vp2", [128, 4, 256])
            for e in range(4):
                nc.tensor.matmul(
                    vp[:, e, 0:192],
                    xT[:, e, 0, :],
                    wA[:, 512:704],
                    start=True,
                    stop=False,
                )
                nc.tensor.matmul(
                    vp[:, e, 0:192],
                    xT[0:65, e, 1, :],
                    wB[:, 512:704],
                    start=False,
                    stop=True,
                )
            nc.scalar.copy(v16[:], vp[:, :, 0:192])

            # ---------------- pair phase: attention core ----------------
            for half in range(2):
                e0 = 2 * half  # elems e0, e0+1 of this quad

                # S scattered per SMAP: [128, bank, slot, 128]
                S = pt("qs3", [128, 3, 4, 128], name=f"S_{half}")
                for ee, h in SORDER:
                    e = e0 + ee
                    r = (h % 3) * 32
                    jq, jk = h // 3, 2 + h // 3
                    b, sl, _ = SMAP[(ee, h)]
                    nc.tensor.matmul(
                        S[:, b, sl, :],
                        qkT[r : r + 32, jq, e, :],
                        qkT[r : r + 32, jk, e, :],
                        start=True,
                        stop=True,
                        tile_position=(r, 0),
                    )

                # P16/Pm/Pn packed [128, 12, 128] in pidx order
                P16 = sbp.tile([128, 12, 128], bf16, tag="P16")
                nc.scalar.activation(
                    P16[:, 0:8, :].rearrange("p (a b) s -> p a b s", a=2),
                    S[:, 0:2, :, :],
                    AF.Exp,
                )
                nc.scalar.activation(P16[:, 8:12, :], S[:, 2, :, :], AF.Exp)

                Pm = sbp.tile([128, 12, 128], bf16, tag="Pm")
                nc.vector.tensor_mul(Pm[:], P16[:], trilR[:])
                rsum = sbp.tile([128, 12], fp32, tag="rsum")
                nc.vector.reduce_sum(rsum[:], Pm[:], axis=mybir.AxisListType.X)
                rrec = sbp.tile([128, 12], fp32, tag="rrec")
                nc.vector.reciprocal(rrec[:], rsum[:])
                rrec2 = sbp.tile([128, 12, 2], bf16, tag="rrec2")
                nc.vector.tensor_copy(
                    rrec2[:], rrec[:, :, None].broadcast_to([128, 12, 2])
                )
                Pn = sbp.tile([128, 12, 128], bf16, tag="Pn")
                nc.vector.tensor_mul(
                    Pn[:].rearrange("p a (c d) -> p a c d", d=2),
                    Pm[:].rearrange("p a (c d) -> p a c d", d=2),
                    rrec2[:, :, None, :].broadcast_to([128, 12, 64, 2]),
                )

                # transposes un-scatter: PT in canonical [h, ee] order
                PT = sbp.tile([128, 6, 2, 128], bf16, tag="PT")
                for ee in range(2):
                    PTp = pt("vp2", [128, 6, 128], bf16, name=f"PTp_{half}_{ee}")
                    for h in range(NH):
                        _, _, pidx = SMAP[(ee, h)]
                        nc.tensor.transpose(
                            PTp[:, h, :], Pn[:, pidx, :], ident[:]
                        )
                    nc.scalar.copy(PT[:, :, ee, :], PTp[:])

                yt = pt("yo1", [128, 2, 2, 128], name=f"yt_{half}")
                for ee in range(2):
                    e = e0 + ee
                    for h in range(NH):
                        r = (h % 4) * 32
                        j = 0 if h < 4 else 1
                        nc.tensor.matmul(
                            yt[r : r + 32, ee, j, :],
                            v16[:, e, h * 32 : h * 32 + 32],
                            PT[:, h, ee, :],
                            start=True,
                            stop=True,
                            tile_position=(0, r),
                        )
                yT = sbp.tile([128, 2, 2, 128], bf16, tag="yT")
                nc.vector.tensor_copy(yT[:, :, 0, :], yt[:, :, 0, :])
                nc.vector.tensor_copy(yT[0:64, :, 1, :], yt[0:64, :, 1, :])
                nc.gpsimd.memset(yT[64:65, :, 1, :], 1.0)

                outs = sbp.tile([128, 2, 192], fp32, tag="outs")
                outp = pt("qs1", [128, 2, 256], name=f"outp_{half}")
                for ee in range(2):
                    nc.tensor.matmul(
                        outp[:, ee, 0:192],
                        yT[:, ee, 0, :],
                        wpA[:],
                        start=True,
                        stop=False,
                    )
                    nc.tensor.matmul(
                        outp[:, ee, 0:192],
                        yT[0:65, ee, 1, :],
                        wpB[:],
                        start=False,
                        stop=True,
                    )
                nc.scalar.copy(outs[:], outp[:, :, 0:192])
                nc.sync.dma_start(
                    out_d[4 * q + e0 : 4 * q + e0 + 2].rearrange(
                        "e t c -> t e c"
                    ),
                    outs[:],
                )

    nc.finalize()
    return nc


def _prep_inputs(x, w_qkv, b_qkv, w_proj, b_proj, bl):
    bf = ml_dtypes.bfloat16
    scale = 1.0 / np.sqrt(HD)
    w2 = np.array(w_qkv, dtype=np.float32, copy=True)
    b2 = np.array(b_qkv, dtype=np.float32, copy=True)
    w2[:, 0:C] *= scale
    b2[0:C] *= scale
    # column order: [q h0-2 +pad | q h3-5 +pad | k h0-2 +pad | k h3-5 +pad | v]
    # (pad cols produce junk in unread partitions, keeping M=128 full-mode)
    perm = np.concatenate(
        [
            np.arange(0, 96),
            np.arange(0, 32),
            np.arange(96, 192),
            np.arange(0, 32),
            np.arange(192, 288),
            np.arange(0, 32),
            np.arange(288, 384),
            np.arange(0, 32),
            np.arange(384, 576),
        ]
    )
    wA = w2[0:128][:, perm].astype(bf)
    wB = np.concatenate([w2[128:192], b2[None, :]], axis=0)[:, perm].astype(bf)
    wpA = np.asarray(w_proj)[0:128].astype(bf)
    wpB = np.concatenate(
        [np.asarray(w_proj)[128:192], np.asarray(b_proj)[None, :]], axis=0
    ).astype(bf)
    trilR = np.ascontiguousarray(
        np.broadcast_to(
            np.tril(np.ones((128, 128), np.float32)), (12, 128, 128)
        ).transpose(1, 0, 2)
    ).astype(bf)
    identR = np.eye(128, dtype=np.float32).astype(bf)
    xs = np.ascontiguousarray(np.asarray(x, dtype=np.float32)).reshape(
        -1, bl, T, C
    )
    maps = []
    for i in range(xs.shape[0]):
        maps.append(
            {
                "x": xs[i],
                "wA": wA,
                "wB": wB,
                "wpA": wpA,
                "wpB": wpB,
                "trilR": trilR,
                "identR": identR,
            }
        )
    return maps


def _run(x, w_qkv, b_qkv, w_proj, b_proj, bl=BL, n_cores=N_CORES, trace=False):
    from concourse.bass_utils import run_bass_kernel_spmd

    key = bl
    if key not in _CACHE:
        _CACHE[key] = _build(bl)
    nc = _CACHE[key]
    maps = _prep_inputs(x, w_qkv, b_qkv, w_proj, b_proj, bl)[:n_cores]
    res = run_bass_kernel_spmd(
        nc, maps, core_ids=list(range(len(maps))), trace=trace
    )
    out = np.concatenate([r["out"] for r in res.results], axis=0)
    return out, res


def kernel(x, w_qkv, b_qkv, w_proj, b_proj):
    out, _ = _run(x, w_qkv, b_qkv, w_proj, b_proj)
    return out.reshape(B, T, C).astype(np.float32)


# revision 27
# speedup vs baseline: 1.4378x; 1.4378x over previous
"""Causal self-attention (B=2048, T=128, C=192, H=6, D=32) on 8 TRN2 cores.

Data-parallel over batch: 256 elems/core. v3: quad-batched qkv (N=512
matmuls), single fused exp per pair, broadcast-AP softmax normalize (one
tensor_tensor instead of 12 tensor_scalars), reduce/copies balanced across
scalar/vector/gpsimd engines.

Per quad (4 elems):
  x --DMA--> xf --cast--> x16 --PE transpose--> xT (+ones row)
  qT/kT = W^T @ xT (N=512, bias fused);  v = xT^T @ Wv (bias fused)
Per pair (2 elems):
  S_h[t,s] = q_h^T k_h (row-tiled PE, 4-concurrent)
  P = exp(S) one ACTIVATE;  Pm = P*tril (bcast mul);  rsum (DVE reduce);
  rrec duplicated-pair recip;  Pn = Pm*rrec_bcast (one mul, 2x mode);
  P^T via PE transpose;  y^T = V^T P^T (col-tiled);  out = y W_p -> HBM.
"""

import sys

sys.path.insert(0, "/opt/trn_rl_repo")

import numpy as np
import ml_dtypes

N_CORES = 8
B, T, C = 2048, 128, 192
NH, HD = 6, 32
BL = B // N_CORES  # 256 per core

_CACHE = {}


def _build(bl):
    from contextlib import ExitStack

    import concourse.bass as bass
    import concourse.mybir as mybir
    import concourse.tile as tile
    from concourse import bacc

    fp32 = mybir.dt.float32
    bf16 = mybir.dt.bfloat16
    AF = mybir.ActivationFunctionType

    nc = bacc.Bacc("TRN2", target_bir_lowering=False, debug=False)

    x_d = nc.dram_tensor("x", [bl, T, C], fp32, kind="ExternalInput")
    wA_d = nc.dram_tensor("wA", [128, 704], bf16, kind="ExternalInput")
    wB_d = nc.dram_tensor("wB", [65, 704], bf16, kind="ExternalInput")
    wpA_d = nc.dram_tensor("wpA", [128, 192], bf16, kind="ExternalInput")
    wpB_d = nc.dram_tensor("wpB", [65, 192], bf16, kind="ExternalInput")
    tril_d = nc.dram_tensor("trilR", [128, 12, 128], bf16, kind="ExternalInput")
    idr_d = nc.dram_tensor("identR", [128, 128], bf16, kind="ExternalInput")
    out_d = nc.dram_tensor("out", [bl, T, C], fp32, kind="ExternalOutput")

    with tile.TileContext(nc) as tc, ExitStack() as ctx:
        consts = ctx.enter_context(tc.tile_pool(name="consts", bufs=1))
        sbq = ctx.enter_context(tc.tile_pool(name="sbq", bufs=2))
        sbp = ctx.enter_context(tc.tile_pool(name="sbp", bufs=4))
        ps = ctx.enter_context(
            tc.tile_pool(name="ps", bufs=1, space=bass.MemorySpace.PSUM)
        )

        wA = consts.tile([128, 704], bf16)
        nc.sync.dma_start(wA[:], wA_d[:])
        wB = consts.tile([65, 704], bf16)
        nc.sync.dma_start(wB[:], wB_d[:])
        wpA = consts.tile([128, 192], bf16)
        nc.sync.dma_start(wpA[:], wpA_d[:])
        wpB = consts.tile([65, 192], bf16)
        nc.sync.dma_start(wpB[:], wpB_d[:])
        trilR = consts.tile([128, 12, 128], bf16)
        nc.sync.dma_start(trilR[:], tril_d[:])
        ident = consts.tile([128, 128], bf16)
        nc.sync.dma_start(ident[:], idr_d[:])

        # q/k packed as 3-head blocks: head h sits at rowgroup 32*(h%3) of
        # j-block h//3. S psum bank = h%3 = the PE row-group, so concurrent
        # row-tiled matmuls always hit distinct banks and same-bank matmuls
        # share a row-group (strictly serialized by the PE). S issue order
        # fills banks 0-1 first so exp1 can start early.
        # pidx: position in the packed [128, 12, 128] P16 layout.
        SORDER = [(0, 0), (0, 1), (0, 3), (0, 4), (1, 0), (1, 1), (1, 3),
                  (1, 4), (0, 2), (0, 5), (1, 2), (1, 5)]
        SMAP = {}
        fill = [0, 0, 0]
        base = [0, 4, 8]
        for ee, h in SORDER:
            b = h % 3
            SMAP[(ee, h)] = (b, fill[b], base[b] + fill[b])
            fill[b] += 1

        def pt(tag, shape, dtype=fp32, name=None):
            return ps.tile(shape, dtype, tag=tag, name=name or f"ps_{tag}")

        for q in range(bl // 4):
            # ---------------- quad phase: load, transpose, qkv ----------
            xf = sbq.tile([128, 4, 192], fp32, tag="xf")
            nc.sync.dma_start(
                xf[:], x_d[4 * q : 4 * q + 4].rearrange("e t c -> t e c")
            )
            x16 = sbq.tile([128, 4, 256], bf16, tag="x16")
            nc.vector.tensor_copy(x16[:, :, 0:192], xf[:])

            xTp = pt("xt1", [128, 4, 2, 128], bf16)
            for e in range(4):
                nc.tensor.transpose(xTp[:, e, 0, :], x16[:, e, 0:128], ident[:])
                nc.tensor.transpose(xTp[:, e, 1, :], x16[:, e, 128:256], ident[:])
            xT = sbq.tile([128, 4, 2, 128], bf16, tag="xT")
            nc.vector.tensor_copy(xT[:], xTp[:])
            nc.gpsimd.memset(xT[64:65, :, 1, :], 1.0)

            # qkT j-blocks [q h0-2 | q h3-5 | k h0-2 | k h3-5] (3 heads + pad
            # per block). j0-j2 -> 3-bank tag shared with S; j3 -> own bank.
            qkT = sbq.tile([128, 4, 4, 128], bf16, tag="qkT")
            T3a = pt("qs3", [128, 3, 4, 128])
            T3b = pt("qs1", [128, 1, 4, 128])
            for j in range(4):
                dst = T3a[:, j, :, :] if j < 3 else T3b[:, 0, :, :]
                nc.tensor.matmul(
                    dst,
                    wA[:, 128 * j : 128 * (j + 1)],
                    xT[:, :, 0, :],
                    start=True,
                    stop=False,
                )
                nc.tensor.matmul(
                    dst,
                    wB[:, 128 * j : 128 * (j + 1)],
                    xT[0:65, :, 1, :],
                    start=False,
                    stop=True,
                )
            nc.scalar.copy(qkT[:, 0:3, :, :], T3a[:])
            nc.scalar.copy(qkT[:, 3, :, :], T3b[:, 0, :, :])

            v16 = sbq.tile([128, 4, 192], bf16, tag="v16")
            vp = pt("vp2", [128, 4, 256])
            for e in range(4):
                nc.tensor.matmul(
                    vp[:, e, 0:192],
                    xT[:, e, 0, :],
                    wA[:, 512:704],
                    start=True,
                    stop=False,
                )
                nc.tensor.matmul(
                    vp[:, e, 0:192],
                    xT[0:65, e, 1, :],
                    wB[:, 512:704],
                    start=False,
                    stop=True,
                )
            nc.scalar.copy(v16[:], vp[:, :, 0:192])

            # ---------------- pair phase: attention core ----------------
            for half in range(2):
                e0 = 2 * half  # elems e0, e0+1 of this quad

                # S scattered per SMAP: [128, bank, slot, 128]
                S = pt("qs3", [128, 3, 4, 128], name=f"S_{half}")
                for ee, h in SORDER:
                    e = e0 + ee
                    r = (h % 3) * 32
                    jq, jk = h // 3, 2 + h // 3
                    b, sl, _ = SMAP[(ee, h)]
                    nc.tensor.matmul(
                        S[:, b, sl, :],
                        qkT[r : r + 32, jq, e, :],
                        qkT[r : r + 32, jk, e, :],
                        start=True,
                        stop=True,
                        tile_position=(r, 0),
                    )

                # P16/Pm/Pn packed [128, 12, 128] in pidx order
                P16 = sbp.tile([128, 12, 128], bf16, tag="P16")
                nc.scalar.activation(
                    P16[:, 0:8, :].rearrange("p (a b) s -> p a b s", a=2),
                    S[:, 0:2, :, :],
                    AF.Exp,
                )
                nc.scalar.activation(P16[:, 8:12, :], S[:, 2, :, :], AF.Exp)

                Pm = sbp.tile([128, 12, 128], bf16, tag="Pm")
                nc.vector.tensor_mul(Pm[:], P16[:], trilR[:])
                rsum = sbp.tile([128, 12], fp32, tag="rsum")
                nc.vector.reduce_sum(rsum[:], Pm[:], axis=mybir.AxisListType.X)
                rrec = sbp.tile([128, 12], fp32, tag="rrec")
                nc.vector.reciprocal(rrec[:], rsum[:])
                rrec2 = sbp.tile([128, 12, 2], bf16, tag="rrec2")
                nc.vector.tensor_copy(
                    rrec2[:], rrec[:, :, None].broadcast_to([128, 12, 2])
                )
                Pn = sbp.tile([128, 12, 128], bf16, tag="Pn")
                nc.vector.tensor_mul(
                    Pn[:].rearrange("p a (c d) -> p a c d", d=2),
                    Pm[:].rearrange("p a (c d) -> p a c d", d=2),
                    rrec2[:, :, None, :].broadcast_to([128, 12, 64, 2]),
                )

                # transposes un-scatter: PT in canonical [h, ee] order
                PT = sbp.tile([128, 6, 2, 128], bf16, tag="PT")
                PTp = pt("vp2", [128, 6, 2, 128], bf16, name=f"PTp_{half}")
                for ee in range(2):
                    for h in range(NH):
                        _, _, pidx = SMAP[(ee, h)]
                        nc.tensor.transpose(
                            PTp[:, h, ee, :], Pn[:, pidx, :], ident[:]
                        )
                nc.scalar.copy(PT[:], PTp[:])

                yt = pt("yo1", [128, 2, 2, 128], name=f"yt_{half}")
                for ee in range(2):
                    e = e0 + ee
                    for h in range(NH):
                        r = (h % 4) * 32
                        j = 0 if h < 4 else 1
                        nc.tensor.matmul(
                            yt[r : r + 32, ee, j, :],
                            v16[:, e, h * 32 : h * 32 + 32],
                            PT[:, h, ee, :],
                            start=True,
                            stop=True,
                            tile_position=(0, r),
                        )
                yT = sbp.tile([128, 2, 2, 128], bf16, tag="yT")
                nc.vector.tensor_copy(yT[:, :, 0, :], yt[:, :, 0, :])
                nc.vector.tensor_copy(yT[0:64, :, 1, :], yt[0:64, :, 1, :])
                nc.gpsimd.memset(yT[64:65, :, 1, :], 1.0)

                outs = sbp.tile([128, 2, 192], fp32, tag="outs")
                outp = pt("yo1", [128, 2, 256], name=f"outp_{half}")
                for ee in range(2):
                    nc.tensor.matmul(
                        outp[:, ee, 0:192],
                        yT[:, ee, 0, :],
                        wpA[:],
                        start=True,
                        stop=False,
                    )
                    nc.tensor.matmul(
                        outp[:, ee, 0:192],
                        yT[0:65, ee, 1, :],
                        wpB[:],
                        start=False,
                        stop=True,
                    )
                nc.scalar.copy(outs[:], outp[:, :, 0:192])
                nc.sync.dma_start(
                    out_d[4 * q + e0 : 4 * q + e0 + 2].rearrange(
                        "e t c -> t e c"
                    ),
                    outs[:],
                )

    nc.finalize()
    return nc


def _prep_inputs(x, w_qkv, b_qkv, w_proj, b_proj, bl):
    bf = ml_dtypes.bfloat16
    scale = 1.0 / np.sqrt(HD)
    w2 = np.array(w_qkv, dtype=np.float32, copy=True)
    b2 = np.array(b_qkv, dtype=np.float32, copy=True)
    w2[:, 0:C] *= scale
    b2[0:C] *= scale
    # column order: [q h0-2 +pad | q h3-5 +pad | k h0-2 +pad | k h3-5 +pad | v]
    # (pad cols produce junk in unread partitions, keeping M=128 full-mode)
    perm = np.concatenate(
        [
            np.arange(0, 96),
            np.arange(0, 32),
            np.arange(96, 192),
            np.arange(0, 32),
            np.arange(192, 288),
            np.arange(0, 32),
            np.arange(288, 384),
            np.arange(0, 32),
            np.arange(384, 576),
        ]
    )
    wA = w2[0:128][:, perm].astype(bf)
    wB = np.concatenate([w2[128:192], b2[None, :]], axis=0)[:, perm].astype(bf)
    wpA = np.asarray(w_proj)[0:128].astype(bf)
    wpB = np.concatenate(
        [np.asarray(w_proj)[128:192], np.asarray(b_proj)[None, :]], axis=0
    ).astype(bf)
    trilR = np.ascontiguousarray(
        np.broadcast_to(
            np.tril(np.ones((128, 128), np.float32)), (12, 128, 128)
        ).transpose(1, 0, 2)
    ).astype(bf)
    identR = np.eye(128, dtype=np.float32).astype(bf)
    xs = np.ascontiguousarray(np.asarray(x, dtype=np.float32)).reshape(
        -1, bl, T, C
    )
    maps = []
    for i in range(xs.shape[0]):
        maps.append(
            {
                "x": xs[i],
                "wA": wA,
                "wB": wB,
                "wpA": wpA,
                "wpB": wpB,
                "trilR": trilR,
                "identR": identR,
            }
        )
    return maps


def _run(x, w_qkv, b_qkv, w_proj, b_proj, bl=BL, n_cores=N_CORES, trace=False):
    from concourse.bass_utils import run_bass_kernel_spmd

    key = bl
    if key not in _CACHE:
        _CACHE[key] = _build(bl)
    nc = _CACHE[key]
    maps = _prep_inputs(x, w_qkv, b_qkv, w_proj, b_proj, bl)[:n_cores]
    res = run_bass_kernel_spmd(
        nc, maps, core_ids=list(range(len(maps))), trace=trace
    )
    out = np.concatenate([r["out"] for r in res.results], axis=0)
    return out, res


def kernel(x, w_qkv, b_qkv, w_proj, b_proj):
    out, _ = _run(x, w_qkv, b_qkv, w_proj, b_proj)
    return out.reshape(B, T, C).astype(np.float32)


# revision 30
# speedup vs baseline: 1.5007x; 1.0438x over previous
"""Causal self-attention (B=2048, T=128, C=192, H=6, D=32) on 8 TRN2 cores.

Data-parallel over batch: 256 elems/core. v3: quad-batched qkv (N=512
matmuls), single fused exp per pair, broadcast-AP softmax normalize (one
tensor_tensor instead of 12 tensor_scalars), reduce/copies balanced across
scalar/vector/gpsimd engines.

Per quad (4 elems):
  x --DMA--> xf --cast--> x16 --PE transpose--> xT (+ones row)
  qT/kT = W^T @ xT (N=512, bias fused);  v = xT^T @ Wv (bias fused)
Per pair (2 elems):
  S_h[t,s] = q_h^T k_h (row-tiled PE, 4-concurrent)
  P = exp(S) one ACTIVATE;  Pm = P*tril (bcast mul);  rsum (DVE reduce);
  rrec duplicated-pair recip;  Pn = Pm*rrec_bcast (one mul, 2x mode);
  P^T via PE transpose;  y^T = V^T P^T (col-tiled);  out = y W_p -> HBM.
"""

import sys

sys.path.insert(0, "/opt/trn_rl_repo")

import numpy as np
import ml_dtypes

N_CORES = 8
B, T, C = 2048, 128, 192
NH, HD = 6, 32
BL = B // N_CORES  # 256 per core

_CACHE = {}


def _build(bl):
    from contextlib import ExitStack

    import concourse.bass as bass
    import concourse.mybir as mybir
    import concourse.tile as tile
    from concourse import bacc

    fp32 = mybir.dt.float32
    bf16 = mybir.dt.bfloat16
    AF = mybir.ActivationFunctionType

    nc = bacc.Bacc("TRN2", target_bir_lowering=False, debug=False)

    x_d = nc.dram_tensor("x", [bl, T, C], fp32, kind="ExternalInput")
    wA_d = nc.dram_tensor("wA", [128, 704], bf16, kind="ExternalInput")
    wB_d = nc.dram_tensor("wB", [65, 704], bf16, kind="ExternalInput")
    wpA_d = nc.dram_tensor("wpA", [128, 192], bf16, kind="ExternalInput")
    wpB_d = nc.dram_tensor("wpB", [65, 192], bf16, kind="ExternalInput")
    tril_d = nc.dram_tensor("trilR", [128, 12, 128], bf16, kind="ExternalInput")
    idr_d = nc.dram_tensor("identR", [128, 128], bf16, kind="ExternalInput")
    out_d = nc.dram_tensor("out", [bl, T, C], fp32, kind="ExternalOutput")

    with tile.TileContext(nc) as tc, ExitStack() as ctx:
        consts = ctx.enter_context(tc.tile_pool(name="consts", bufs=1))
        sbq = ctx.enter_context(tc.tile_pool(name="sbq", bufs=3))
        sbp = ctx.enter_context(tc.tile_pool(name="sbp", bufs=4))
        ps = ctx.enter_context(
            tc.tile_pool(name="ps", bufs=1, space=bass.MemorySpace.PSUM)
        )

        wA = consts.tile([128, 704], bf16)
        nc.sync.dma_start(wA[:], wA_d[:])
        wB = consts.tile([65, 704], bf16)
        nc.sync.dma_start(wB[:], wB_d[:])
        wpA = consts.tile([128, 192], bf16)
        nc.sync.dma_start(wpA[:], wpA_d[:])
        wpB = consts.tile([65, 192], bf16)
        nc.sync.dma_start(wpB[:], wpB_d[:])
        trilR = consts.tile([128, 12, 128], bf16)
        nc.sync.dma_start(trilR[:], tril_d[:])
        ident = consts.tile([128, 128], bf16)
        nc.sync.dma_start(ident[:], idr_d[:])

        # q/k packed as 3-head blocks: head h sits at rowgroup 32*(h%3) of
        # j-block h//3. S psum bank = h%3 = the PE row-group, so concurrent
        # row-tiled matmuls always hit distinct banks and same-bank matmuls
        # share a row-group (strictly serialized by the PE). S issue order
        # fills banks 0-1 first so exp1 can start early.
        # pidx: position in the packed [128, 12, 128] P16 layout.
        SORDER = [(0, 0), (0, 1), (0, 3), (0, 4), (1, 0), (1, 1), (1, 3),
                  (1, 4), (0, 2), (0, 5), (1, 2), (1, 5)]
        SMAP = {}
        fill = [0, 0, 0]
        base = [0, 4, 8]
        for ee, h in SORDER:
            b = h % 3
            SMAP[(ee, h)] = (b, fill[b], base[b] + fill[b])
            fill[b] += 1

        def pt(tag, shape, dtype=fp32, name=None):
            return ps.tile(shape, dtype, tag=tag, name=name or f"ps_{tag}")

        for q in range(bl // 4):
            # ---------------- quad phase: load, transpose, qkv ----------
            xf = sbq.tile([128, 4, 192], fp32, tag="xf")
            nc.sync.dma_start(
                xf[:], x_d[4 * q : 4 * q + 4].rearrange("e t c -> t e c")
            )
            x16 = sbq.tile([128, 4, 256], bf16, tag="x16")
            nc.vector.tensor_copy(x16[:, :, 0:192], xf[:])

            xTp = pt("xt1", [128, 4, 2, 128], bf16)
            for e in range(4):
                nc.tensor.transpose(xTp[:, e, 0, :], x16[:, e, 0:128], ident[:])
                nc.tensor.transpose(xTp[:, e, 1, :], x16[:, e, 128:256], ident[:])
            xT = sbq.tile([128, 4, 2, 128], bf16, tag="xT")
            nc.vector.tensor_copy(xT[:], xTp[:])
            nc.gpsimd.memset(xT[64:65, :, 1, :], 1.0)

            # qkT j-blocks [q h0-2 | q h3-5 | k h0-2 | k h3-5] (3 heads + pad
            # per block). j0-j2 -> 3-bank tag shared with S; j3 -> own bank.
            qkT = sbq.tile([128, 4, 4, 128], bf16, tag="qkT")
            T3a = pt("qs3", [128, 3, 4, 128])
            T3b = pt("qs1", [128, 1, 4, 128])
            for j in range(4):
                dst = T3a[:, j, :, :] if j < 3 else T3b[:, 0, :, :]
                nc.tensor.matmul(
                    dst,
                    wA[:, 128 * j : 128 * (j + 1)],
                    xT[:, :, 0, :],
                    start=True,
                    stop=False,
                )
                nc.tensor.matmul(
                    dst,
                    wB[:, 128 * j : 128 * (j + 1)],
                    xT[0:65, :, 1, :],
                    start=False,
                    stop=True,
                )
            nc.scalar.copy(qkT[:, 0:3, :, :], T3a[:])
            nc.scalar.copy(qkT[:, 3, :, :], T3b[:, 0, :, :])

            v16 = sbq.tile([128, 4, 192], bf16, tag="v16")
            vp = pt("vp2", [128, 4, 256])
            for e in range(4):
                nc.tensor.matmul(
                    vp[:, e, 0:192],
                    xT[:, e, 0, :],
                    wA[:, 512:704],
                    start=True,
                    stop=False,
                )
                nc.tensor.matmul(
                    vp[:, e, 0:192],
                    xT[0:65, e, 1, :],
                    wB[:, 512:704],
                    start=False,
                    stop=True,
                )
            nc.scalar.copy(v16[:], vp[:, :, 0:192])

            # ---------------- pair phase: attention core ----------------
            for half in range(2):
                e0 = 2 * half  # elems e0, e0+1 of this quad

                # S scattered per SMAP: [128, bank, slot, 128]
                S = pt("qs3", [128, 3, 4, 128], name=f"S_{half}")
                for ee, h in SORDER:
                    e = e0 + ee
                    r = (h % 3) * 32
                    jq, jk = h // 3, 2 + h // 3
                    b, sl, _ = SMAP[(ee, h)]
                    nc.tensor.matmul(
                        S[:, b, sl, :],
                        qkT[r : r + 32, jq, e, :],
                        qkT[r : r + 32, jk, e, :],
                        start=True,
                        stop=True,
                        tile_position=(r, 0),
                    )

                # P16/Pm/Pn packed [128, 12, 128] in pidx order
                P16 = sbp.tile([128, 12, 128], bf16, tag="P16")
                nc.scalar.activation(
                    P16[:, 0:8, :].rearrange("p (a b) s -> p a b s", a=2),
                    S[:, 0:2, :, :],
                    AF.Exp,
                )
                nc.scalar.activation(P16[:, 8:12, :], S[:, 2, :, :], AF.Exp)

                Pm = sbp.tile([128, 12, 128], bf16, tag="Pm")
                nc.vector.tensor_mul(Pm[:], P16[:], trilR[:])
                rsum = sbp.tile([128, 12], fp32, tag="rsum")
                nc.vector.reduce_sum(rsum[:], Pm[:], axis=mybir.AxisListType.X)
                rrec = sbp.tile([128, 12], fp32, tag="rrec")
                nc.vector.reciprocal(rrec[:], rsum[:])
                rrec2 = sbp.tile([128, 12, 2], bf16, tag="rrec2")
                nc.vector.tensor_copy(
                    rrec2[:], rrec[:, :, None].broadcast_to([128, 12, 2])
                )
                Pn = sbp.tile([128, 12, 128], bf16, tag="Pn")
                nc.vector.tensor_mul(
                    Pn[:].rearrange("p a (c d) -> p a c d", d=2),
                    Pm[:].rearrange("p a (c d) -> p a c d", d=2),
                    rrec2[:, :, None, :].broadcast_to([128, 12, 64, 2]),
                )

                # transposes un-scatter: PT in canonical [h, ee] order
                PT = sbp.tile([128, 6, 2, 128], bf16, tag="PT")
                PTp = pt("vp2", [128, 6, 2, 128], bf16, name=f"PTp_{half}")
                for ee in range(2):
                    for h in range(NH):
                        _, _, pidx = SMAP[(ee, h)]
                        nc.tensor.transpose(
                            PTp[:, h, ee, :], Pn[:, pidx, :], ident[:]
                        )
                    nc.scalar.copy(PT[:, :, ee, :], PTp[:, :, ee, :])

                yt = pt("yo1", [128, 2, 2, 128], name=f"yt_{half}")
                for ee in range(2):
                    e = e0 + ee
                    for h in range(NH):
                        r = (h % 4) * 32
                        j = 0 if h < 4 else 1
                        nc.tensor.matmul(
                            yt[r : r + 32, ee, j, :],
                            v16[:, e, h * 32 : h * 32 + 32],
                            PT[:, h, ee, :],
                            start=True,
                            stop=True,
                            tile_position=(0, r),
                        )
                yT = sbp.tile([128, 2, 2, 128], bf16, tag="yT")
                nc.vector.tensor_copy(yT[:, :, 0, :], yt[:, :, 0, :])
                nc.vector.tensor_copy(yT[0:64, :, 1, :], yt[0:64, :, 1, :])
                nc.gpsimd.memset(yT[64:65, :, 1, :], 1.0)

                outs = sbp.tile([128, 2, 192], fp32, tag="outs")
                outp = pt("yo1", [128, 2, 256], name=f"outp_{half}")
                for ee in range(2):
                    nc.tensor.matmul(
                        outp[:, ee, 0:192],
                        yT[:, ee, 0, :],
                        wpA[:],
                        start=True,
                        stop=False,
                    )
                    nc.tensor.matmul(
                        outp[:, ee, 0:192],
                        yT[0:65, ee, 1, :],
                        wpB[:],
                        start=False,
                        stop=True,
                    )
                    nc.scalar.copy(outs[:, ee, :], outp[:, ee, 0:192])
                nc.sync.dma_start(
                    out_d[4 * q + e0 : 4 * q + e0 + 2].rearrange(
                        "e t c -> t e c"
                    ),
                    outs[:],
                )

    nc.finalize()
    return nc


def _prep_inputs(x, w_qkv, b_qkv, w_proj, b_proj, bl):
    bf = ml_dtypes.bfloat16
    scale = 1.0 / np.sqrt(HD)
    w2 = np.array(w_qkv, dtype=np.float32, copy=True)
    b2 = np.array(b_qkv, dtype=np.float32, copy=True)
    w2[:, 0:C] *= scale
    b2[0:C] *= scale
    # column order: [q h0-2 +pad | q h3-5 +pad | k h0-2 +pad | k h3-5 +pad | v]
    # (pad cols produce junk in unread partitions, keeping M=128 full-mode)
    perm = np.concatenate(
        [
            np.arange(0, 96),
            np.arange(0, 32),
            np.arange(96, 192),
            np.arange(0, 32),
            np.arange(192, 288),
            np.arange(0, 32),
            np.arange(288, 384),
            np.arange(0, 32),
            np.arange(384, 576),
        ]
    )
    wA = w2[0:128][:, perm].astype(bf)
    wB = np.concatenate([w2[128:192], b2[None, :]], axis=0)[:, perm].astype(bf)
    wpA = np.asarray(w_proj)[0:128].astype(bf)
    wpB = np.concatenate(
        [np.asarray(w_proj)[128:192], np.asarray(b_proj)[None, :]], axis=0
    ).astype(bf)
    trilR = np.ascontiguousarray(
        np.broadcast_to(
            np.tril(np.ones((128, 128), np.float32)), (12, 128, 128)
        ).transpose(1, 0, 2)
    ).astype(bf)
    identR = np.eye(128, dtype=np.float32).astype(bf)
    xs = np.ascontiguousarray(np.asarray(x, dtype=np.float32)).reshape(
        -1, bl, T, C
    )
    maps = []
    for i in range(xs.shape[0]):
        maps.append(
            {
                "x": xs[i],
                "wA": wA,
                "wB": wB,
                "wpA": wpA,
                "wpB": wpB,
                "trilR": trilR,
                "identR": identR,
            }
        )
    return maps


def _run(x, w_qkv, b_qkv, w_proj, b_proj, bl=BL, n_cores=N_CORES, trace=False):
    from concourse.bass_utils import run_bass_kernel_spmd

    key = bl
    if key not in _CACHE:
        _CACHE[key] = _build(bl)
    nc = _CACHE[key]
    maps = _prep_inputs(x, w_qkv, b_qkv, w_proj, b_proj, bl)[:n_cores]
    res = run_bass_kernel_spmd(
        nc, maps, core_ids=list(range(len(maps))), trace=trace
    )
    out = np.concatenate([r["out"] for r in res.results], axis=0)
    return out, res


def kernel(x, w_qkv, b_qkv, w_proj, b_proj):
    out, _ = _run(x, w_qkv, b_qkv, w_proj, b_proj)
    return out.reshape(B, T, C).astype(np.float32)


# revision 32
# speedup vs baseline: 1.7380x; 1.1581x over previous
"""Causal self-attention (B=2048, T=128, C=192, H=6, D=32) on 8 TRN2 cores.

Data-parallel over batch: 256 elems/core. v3: quad-batched qkv (N=512
matmuls), single fused exp per pair, broadcast-AP softmax normalize (one
tensor_tensor instead of 12 tensor_scalars), reduce/copies balanced across
scalar/vector/gpsimd engines.

Per quad (4 elems):
  x --DMA--> xf --cast--> x16 --PE transpose--> xT (+ones row)
  qT/kT = W^T @ xT (N=512, bias fused);  v = xT^T @ Wv (bias fused)
Per pair (2 elems):
  S_h[t,s] = q_h^T k_h (row-tiled PE, 4-concurrent)
  P = exp(S) one ACTIVATE;  Pm = P*tril (bcast mul);  rsum (DVE reduce);
  rrec duplicated-pair recip;  Pn = Pm*rrec_bcast (one mul, 2x mode);
  P^T via PE transpose;  y^T = V^T P^T (col-tiled);  out = y W_p -> HBM.
"""

import sys

sys.path.insert(0, "/opt/trn_rl_repo")

import numpy as np
import ml_dtypes

N_CORES = 8
B, T, C = 2048, 128, 192
NH, HD = 6, 32
BL = B // N_CORES  # 256 per core

_CACHE = {}


def _build(bl):
    from contextlib import ExitStack

    import concourse.bass as bass
    import concourse.mybir as mybir
    import concourse.tile as tile
    from concourse import bacc

    fp32 = mybir.dt.float32
    bf16 = mybir.dt.bfloat16
    AF = mybir.ActivationFunctionType

    nc = bacc.Bacc("TRN2", target_bir_lowering=False, debug=False)

    x_d = nc.dram_tensor("x", [bl, T, C], fp32, kind="ExternalInput")
    wA_d = nc.dram_tensor("wA", [128, 704], bf16, kind="ExternalInput")
    wB_d = nc.dram_tensor("wB", [65, 704], bf16, kind="ExternalInput")
    wpA_d = nc.dram_tensor("wpA", [128, 192], bf16, kind="ExternalInput")
    wpB_d = nc.dram_tensor("wpB", [65, 192], bf16, kind="ExternalInput")
    tril_d = nc.dram_tensor("trilR", [128, 12, 128], bf16, kind="ExternalInput")
    idr_d = nc.dram_tensor("identR", [128, 128], bf16, kind="ExternalInput")
    out_d = nc.dram_tensor("out", [bl, T, C], fp32, kind="ExternalOutput")

    with tile.TileContext(nc) as tc, ExitStack() as ctx:
        consts = ctx.enter_context(tc.tile_pool(name="consts", bufs=1))
        sbq = ctx.enter_context(tc.tile_pool(name="sbq", bufs=3))
        sbp = ctx.enter_context(tc.tile_pool(name="sbp", bufs=4))
        ps = ctx.enter_context(
            tc.tile_pool(name="ps", bufs=1, space=bass.MemorySpace.PSUM)
        )

        wA = consts.tile([128, 704], bf16)
        nc.sync.dma_start(wA[:], wA_d[:])
        wB = consts.tile([65, 704], bf16)
        nc.sync.dma_start(wB[:], wB_d[:])
        wpA = consts.tile([128, 192], bf16)
        nc.sync.dma_start(wpA[:], wpA_d[:])
        wpB = consts.tile([65, 192], bf16)
        nc.sync.dma_start(wpB[:], wpB_d[:])
        trilR = consts.tile([128, 12, 128], bf16)
        nc.sync.dma_start(trilR[:], tril_d[:])
        ident = consts.tile([128, 128], bf16)
        nc.sync.dma_start(ident[:], idr_d[:])

        # q/k packed as 3-head blocks: head h sits at rowgroup 32*(h%3) of
        # j-block h//3. S psum bank = h%3 = the PE row-group, so concurrent
        # row-tiled matmuls always hit distinct banks and same-bank matmuls
        # share a row-group (strictly serialized by the PE). S issue order
        # fills banks 0-1 first so exp1 can start early.
        # pidx: position in the packed [128, 12, 128] P16 layout.
        # ee-major issue order: ee0 fills slots 0-1, ee1 slots 2-3 of each
        # bank, so each ee's exp/softmax chain runs on a clean rectangle and
        # ee0's downstream overlaps ee1's S matmuls + exp.
        SORDER = [(0, 0), (0, 1), (0, 3), (0, 4), (0, 2), (0, 5),
                  (1, 0), (1, 1), (1, 3), (1, 4), (1, 2), (1, 5)]
        SMAP = {}
        fill = [0, 0, 0]
        for ee, h in SORDER:
            b = h % 3
            SMAP[(ee, h)] = (b, fill[b], 6 * ee + 2 * b + (fill[b] % 2))
            fill[b] += 1

        def pt(tag, shape, dtype=fp32, name=None):
            return ps.tile(shape, dtype, tag=tag, name=name or f"ps_{tag}")

        for q in range(bl // 4):
            # ---------------- quad phase: load, transpose, qkv ----------
            xf = sbq.tile([128, 4, 192], fp32, tag="xf")
            nc.sync.dma_start(
                xf[:], x_d[4 * q : 4 * q + 4].rearrange("e t c -> t e c")
            )
            x16 = sbq.tile([128, 4, 256], bf16, tag="x16")
            nc.vector.tensor_copy(x16[:, :, 0:192], xf[:])

            xTp = pt("xt1", [128, 4, 2, 128], bf16)
            for e in range(4):
                nc.tensor.transpose(xTp[:, e, 0, :], x16[:, e, 0:128], ident[:])
                nc.tensor.transpose(xTp[:, e, 1, :], x16[:, e, 128:256], ident[:])
            xT = sbq.tile([128, 4, 2, 128], bf16, tag="xT")
            nc.vector.tensor_copy(xT[:], xTp[:])
            nc.gpsimd.memset(xT[64:65, :, 1, :], 1.0)

            # qkT j-blocks [q h0-2 | q h3-5 | k h0-2 | k h3-5] (3 heads + pad
            # per block). j0-j2 -> 3-bank tag shared with S; j3 -> own bank.
            qkT = sbq.tile([128, 4, 4, 128], bf16, tag="qkT")
            T3a = pt("qs3", [128, 3, 4, 128])
            T3b = pt("qs1", [128, 1, 4, 128])
            for j in range(4):
                dst = T3a[:, j, :, :] if j < 3 else T3b[:, 0, :, :]
                nc.tensor.matmul(
                    dst,
                    wA[:, 128 * j : 128 * (j + 1)],
                    xT[:, :, 0, :],
                    start=True,
                    stop=False,
                )
                nc.tensor.matmul(
                    dst,
                    wB[:, 128 * j : 128 * (j + 1)],
                    xT[0:65, :, 1, :],
                    start=False,
                    stop=True,
                )
            nc.scalar.copy(qkT[:, 0:3, :, :], T3a[:])
            nc.scalar.copy(qkT[:, 3, :, :], T3b[:, 0, :, :])

            v16 = sbq.tile([128, 4, 192], bf16, tag="v16")
            vp = pt("vp2", [128, 4, 256])
            for e in range(4):
                nc.tensor.matmul(
                    vp[:, e, 0:192],
                    xT[:, e, 0, :],
                    wA[:, 512:704],
                    start=True,
                    stop=False,
                )
                nc.tensor.matmul(
                    vp[:, e, 0:192],
                    xT[0:65, e, 1, :],
                    wB[:, 512:704],
                    start=False,
                    stop=True,
                )
            nc.scalar.copy(v16[:], vp[:, :, 0:192])

            # ---------------- pair phase: attention core ----------------
            for half in range(2):
                e0 = 2 * half  # elems e0, e0+1 of this quad

                # S scattered per SMAP: [128, bank, slot, 128]
                S = pt("qs3", [128, 3, 4, 128], name=f"S_{half}")
                for ee, h in SORDER:
                    e = e0 + ee
                    r = (h % 3) * 32
                    jq, jk = h // 3, 2 + h // 3
                    b, sl, _ = SMAP[(ee, h)]
                    nc.tensor.matmul(
                        S[:, b, sl, :],
                        qkT[r : r + 32, jq, e, :],
                        qkT[r : r + 32, jk, e, :],
                        start=True,
                        stop=True,
                        tile_position=(r, 0),
                    )

                # P16/Pm/Pn packed [128, 12, 128] in pidx order (ee-major);
                # full softmax chain split per ee for latency overlap
                P16 = sbp.tile([128, 12, 128], bf16, tag="P16")
                Pm = sbp.tile([128, 12, 128], bf16, tag="Pm")
                rsum = sbp.tile([128, 12], fp32, tag="rsum")
                rrec = sbp.tile([128, 12], fp32, tag="rrec")
                rrec2 = sbp.tile([128, 12, 2], bf16, tag="rrec2")
                Pn = sbp.tile([128, 12, 128], bf16, tag="Pn")
                PT = sbp.tile([128, 6, 2, 128], bf16, tag="PT")
                PTp = pt("vp2", [128, 6, 2, 128], bf16, name=f"PTp_{half}")
                for ee in range(2):
                    lo = 6 * ee
                    nc.scalar.activation(
                        P16[:, lo : lo + 6, :].rearrange(
                            "p (a b) s -> p a b s", a=3
                        ),
                        S[:, :, 2 * ee : 2 * ee + 2, :],
                        AF.Exp,
                    )
                    nc.vector.tensor_mul(
                        Pm[:, lo : lo + 6, :],
                        P16[:, lo : lo + 6, :],
                        trilR[:, 0:6, :],
                    )
                    nc.vector.reduce_sum(
                        rsum[:, lo : lo + 6],
                        Pm[:, lo : lo + 6, :],
                        axis=mybir.AxisListType.X,
                    )
                    nc.vector.reciprocal(
                        rrec[:, lo : lo + 6], rsum[:, lo : lo + 6]
                    )
                    nc.vector.tensor_copy(
                        rrec2[:, lo : lo + 6, :],
                        rrec[:, lo : lo + 6, None].broadcast_to([128, 6, 2]),
                    )
                    nc.vector.tensor_mul(
                        Pn[:, lo : lo + 6, :].rearrange(
                            "p a (c d) -> p a c d", d=2
                        ),
                        Pm[:, lo : lo + 6, :].rearrange(
                            "p a (c d) -> p a c d", d=2
                        ),
                        rrec2[:, lo : lo + 6, None, :].broadcast_to(
                            [128, 6, 64, 2]
                        ),
                    )
                    for h in range(NH):
                        _, _, pidx = SMAP[(ee, h)]
                        nc.tensor.transpose(
                            PTp[:, h, ee, :], Pn[:, pidx, :], ident[:]
                        )
                    nc.scalar.copy(PT[:, :, ee, :], PTp[:, :, ee, :])

                yt = pt("yo1", [128, 2, 2, 128], name=f"yt_{half}")
                for ee in range(2):
                    e = e0 + ee
                    for h in range(NH):
                        r = (h % 4) * 32
                        j = 0 if h < 4 else 1
                        nc.tensor.matmul(
                            yt[r : r + 32, ee, j, :],
                            v16[:, e, h * 32 : h * 32 + 32],
                            PT[:, h, ee, :],
                            start=True,
                            stop=True,
                            tile_position=(0, r),
                        )
                yT = sbp.tile([128, 2, 2, 128], bf16, tag="yT")
                nc.vector.tensor_copy(yT[:, :, 0, :], yt[:, :, 0, :])
                nc.vector.tensor_copy(yT[0:64, :, 1, :], yt[0:64, :, 1, :])
                nc.gpsimd.memset(yT[64:65, :, 1, :], 1.0)

                outs = sbp.tile([128, 2, 192], fp32, tag="outs")
                outp = pt("yo1", [128, 2, 256], name=f"outp_{half}")
                for ee in range(2):
                    nc.tensor.matmul(
                        outp[:, ee, 0:192],
                        yT[:, ee, 0, :],
                        wpA[:],
                        start=True,
                        stop=False,
                    )
                    nc.tensor.matmul(
                        outp[:, ee, 0:192],
                        yT[0:65, ee, 1, :],
                        wpB[:],
                        start=False,
                        stop=True,
                    )
                    nc.scalar.copy(outs[:, ee, :], outp[:, ee, 0:192])
                nc.sync.dma_start(
                    out_d[4 * q + e0 : 4 * q + e0 + 2].rearrange(
                        "e t c -> t e c"
                    ),
                    outs[:],
                )

    nc.finalize()
    return nc


def _prep_inputs(x, w_qkv, b_qkv, w_proj, b_proj, bl):
    bf = ml_dtypes.bfloat16
    scale = 1.0 / np.sqrt(HD)
    w2 = np.array(w_qkv, dtype=np.float32, copy=True)
    b2 = np.array(b_qkv, dtype=np.float32, copy=True)
    w2[:, 0:C] *= scale
    b2[0:C] *= scale
    # column order: [q h0-2 +pad | q h3-5 +pad | k h0-2 +pad | k h3-5 +pad | v]
    # (pad cols produce junk in unread partitions, keeping M=128 full-mode)
    perm = np.concatenate(
        [
            np.arange(0, 96),
            np.arange(0, 32),
            np.arange(96, 192),
            np.arange(0, 32),
            np.arange(192, 288),
            np.arange(0, 32),
            np.arange(288, 384),
            np.arange(0, 32),
            np.arange(384, 576),
        ]
    )
    wA = w2[0:128][:, perm].astype(bf)
    wB = np.concatenate([w2[128:192], b2[None, :]], axis=0)[:, perm].astype(bf)
    wpA = np.asarray(w_proj)[0:128].astype(bf)
    wpB = np.concatenate(
        [np.asarray(w_proj)[128:192], np.asarray(b_proj)[None, :]], axis=0
    ).astype(bf)
    trilR = np.ascontiguousarray(
        np.broadcast_to(
            np.tril(np.ones((128, 128), np.float32)), (12, 128, 128)
        ).transpose(1, 0, 2)
    ).astype(bf)
    identR = np.eye(128, dtype=np.float32).astype(bf)
    xs = np.ascontiguousarray(np.asarray(x, dtype=np.float32)).reshape(
        -1, bl, T, C
    )
    maps = []
    for i in range(xs.shape[0]):
        maps.append(
            {
                "x": xs[i],
                "wA": wA,
                "wB": wB,
                "wpA": wpA,
                "wpB": wpB,
                "trilR": trilR,
                "identR": identR,
            }
        )
    return maps


def _run(x, w_qkv, b_qkv, w_proj, b_proj, bl=BL, n_cores=N_CORES, trace=False):
    from concourse.bass_utils import run_bass_kernel_spmd

    key = bl
    if key not in _CACHE:
        _CACHE[key] = _build(bl)
    nc = _CACHE[key]
    maps = _prep_inputs(x, w_qkv, b_qkv, w_proj, b_proj, bl)[:n_cores]
    res = run_bass_kernel_spmd(
        nc, maps, core_ids=list(range(len(maps))), trace=trace
    )
    out = np.concatenate([r["out"] for r in res.results], axis=0)
    return out, res


def kernel(x, w_qkv, b_qkv, w_proj, b_proj):
    out, _ = _run(x, w_qkv, b_qkv, w_proj, b_proj)
    return out.reshape(B, T, C).astype(np.float32)


# revision 40
# speedup vs baseline: 1.7762x; 1.0220x over previous
"""Causal self-attention (B=2048, T=128, C=192, H=6, D=32) on 8 TRN2 cores.

Data-parallel over batch: 256 elems/core. v3: quad-batched qkv (N=512
matmuls), single fused exp per pair, broadcast-AP softmax normalize (one
tensor_tensor instead of 12 tensor_scalars), reduce/copies balanced across
scalar/vector/gpsimd engines.

Per quad (4 elems):
  x --DMA--> xf --cast--> x16 --PE transpose--> xT (+ones row)
  qT/kT = W^T @ xT (N=512, bias fused);  v = xT^T @ Wv (bias fused)
Per pair (2 elems):
  S_h[t,s] = q_h^T k_h (row-tiled PE, 4-concurrent)
  P = exp(S) one ACTIVATE;  Pm = P*tril (bcast mul);  rsum (DVE reduce);
  rrec duplicated-pair recip;  Pn = Pm*rrec_bcast (one mul, 2x mode);
  P^T via PE transpose;  y^T = V^T P^T (col-tiled);  out = y W_p -> HBM.
"""

import sys

sys.path.insert(0, "/opt/trn_rl_repo")

import numpy as np
import ml_dtypes

N_CORES = 8
B, T, C = 2048, 128, 192
NH, HD = 6, 32
BL = B // N_CORES  # 256 per core

_CACHE = {}


def _build(bl):
    from contextlib import ExitStack

    import concourse.bass as bass
    import concourse.mybir as mybir
    import concourse.tile as tile
    from concourse import bacc

    fp32 = mybir.dt.float32
    bf16 = mybir.dt.bfloat16
    AF = mybir.ActivationFunctionType

    nc = bacc.Bacc("TRN2", target_bir_lowering=False, debug=False)

    x_d = nc.dram_tensor("x", [bl, T, C], fp32, kind="ExternalInput")
    wA_d = nc.dram_tensor("wA", [128, 704], bf16, kind="ExternalInput")
    wB_d = nc.dram_tensor("wB", [65, 704], bf16, kind="ExternalInput")
    wpA_d = nc.dram_tensor("wpA", [128, 192], bf16, kind="ExternalInput")
    wpB_d = nc.dram_tensor("wpB", [65, 192], bf16, kind="ExternalInput")
    tril_d = nc.dram_tensor("trilR", [128, 12, 128], bf16, kind="ExternalInput")
    idr_d = nc.dram_tensor("identR", [128, 128], bf16, kind="ExternalInput")
    idf_d = nc.dram_tensor("identF", [128, 128], fp32, kind="ExternalInput")
    out_d = nc.dram_tensor("out", [bl, T, C], fp32, kind="ExternalOutput")

    with tile.TileContext(nc) as tc, ExitStack() as ctx:
        consts = ctx.enter_context(tc.tile_pool(name="consts", bufs=1))
        sbq = ctx.enter_context(tc.tile_pool(name="sbq", bufs=3))
        sbp = ctx.enter_context(tc.tile_pool(name="sbp", bufs=5))
        ps = ctx.enter_context(
            tc.tile_pool(name="ps", bufs=1, space=bass.MemorySpace.PSUM)
        )

        wA = consts.tile([128, 704], bf16)
        nc.sync.dma_start(wA[:], wA_d[:])
        wB = consts.tile([65, 704], bf16)
        nc.sync.dma_start(wB[:], wB_d[:])
        wpA = consts.tile([128, 192], bf16)
        nc.sync.dma_start(wpA[:], wpA_d[:])
        wpB = consts.tile([65, 192], bf16)
        nc.sync.dma_start(wpB[:], wpB_d[:])
        trilR = consts.tile([128, 12, 128], bf16)
        nc.sync.dma_start(trilR[:], tril_d[:])
        ident = consts.tile([128, 128], bf16)
        nc.sync.dma_start(ident[:], idr_d[:])
        identf = consts.tile([128, 128], fp32)
        nc.sync.dma_start(identf[:], idf_d[:])

        # q/k packed as 3-head blocks: head h sits at rowgroup 32*(h%3) of
        # j-block h//3. S psum bank = h%3 = the PE row-group, so concurrent
        # row-tiled matmuls always hit distinct banks and same-bank matmuls
        # share a row-group (strictly serialized by the PE). S issue order
        # fills banks 0-1 first so exp1 can start early.
        # pidx: position in the packed [128, 12, 128] P16 layout.
        # ee-major issue order: ee0 fills slots 0-1, ee1 slots 2-3 of each
        # bank, so each ee's exp/softmax chain runs on a clean rectangle and
        # ee0's downstream overlaps ee1's S matmuls + exp.
        SORDER = [(0, 0), (0, 1), (0, 3), (0, 4), (0, 2), (0, 5),
                  (1, 0), (1, 1), (1, 3), (1, 4), (1, 2), (1, 5)]
        SMAP = {}
        fill = [0, 0, 0]
        for ee, h in SORDER:
            b = h % 3
            SMAP[(ee, h)] = (b, fill[b], 6 * ee + 2 * b + (fill[b] % 2))
            fill[b] += 1

        def pt(tag, shape, dtype=fp32, name=None):
            return ps.tile(shape, dtype, tag=tag, name=name or f"ps_{tag}")

        for q in range(bl // 4):
            # ---------------- quad phase: load, transpose, qkv ----------
            xf = sbq.tile([128, 4, 192], fp32, tag="xf")
            nc.sync.dma_start(
                xf[:], x_d[4 * q : 4 * q + 4].rearrange("e t c -> t e c")
            )
            # transpose x in fp32 directly (transpose-mode is fast for fp32);
            # cast to bf16 during psum eviction
            xT = sbq.tile([128, 4, 2, 128], bf16, tag="xT")
            xTp0 = pt("xt1", [128, 4, 128], fp32, name="xTp0")
            for e in range(4):
                nc.tensor.transpose(xTp0[:, e, :], xf[:, e, 0:128], identf[:])
            nc.vector.tensor_copy(xT[:, :, 0, :], xTp0[:])
            xTp1 = pt("xt1", [128, 4, 128], fp32, name="xTp1")
            for e in range(4):
                nc.tensor.transpose(
                    xTp1[0:64, e, :], xf[:, e, 128:192], identf[:]
                )
            nc.vector.tensor_copy(xT[0:64, :, 1, :], xTp1[0:64, :, :])
            nc.gpsimd.memset(xT[64:65, :, 1, :], 1.0)

            # qkT j-blocks [q h0-2 | q h3-5 | k h0-2 | k h3-5] (3 heads + pad
            # per block). j0-j2 -> 3-bank tag shared with S; j3 -> own bank.
            qkT = sbq.tile([128, 4, 4, 128], bf16, tag="qkT")
            T3a = pt("qs3", [128, 3, 4, 128])
            T3b = pt("qs1", [128, 1, 4, 128])
            for j in range(4):
                dst = T3a[:, j, :, :] if j < 3 else T3b[:, 0, :, :]
                nc.tensor.matmul(
                    dst,
                    wA[:, 128 * j : 128 * (j + 1)],
                    xT[:, :, 0, :],
                    start=True,
                    stop=False,
                )
                nc.tensor.matmul(
                    dst,
                    wB[:, 128 * j : 128 * (j + 1)],
                    xT[0:65, :, 1, :],
                    start=False,
                    stop=True,
                )
            nc.scalar.copy(qkT[:, 0:3, :, :], T3a[:])
            nc.scalar.copy(qkT[:, 3, :, :], T3b[:, 0, :, :])

            v16 = sbq.tile([128, 4, 192], bf16, tag="v16")
            vp = pt("vp2", [128, 4, 256])
            for e in range(4):
                nc.tensor.matmul(
                    vp[:, e, 0:192],
                    xT[:, e, 0, :],
                    wA[:, 512:704],
                    start=True,
                    stop=False,
                )
                nc.tensor.matmul(
                    vp[:, e, 0:192],
                    xT[0:65, e, 1, :],
                    wB[:, 512:704],
                    start=False,
                    stop=True,
                )
            nc.scalar.copy(v16[:], vp[:, :, 0:192])

            # ---------------- pair phase: attention core ----------------
            for half in range(2):
                e0 = 2 * half  # elems e0, e0+1 of this quad

                # S scattered per SMAP: [128, bank, slot, 128]
                S = pt("qs3", [128, 3, 4, 128], name=f"S_{half}")
                for ee, h in SORDER:
                    e = e0 + ee
                    r = (h % 3) * 32
                    jq, jk = h // 3, 2 + h // 3
                    b, sl, _ = SMAP[(ee, h)]
                    nc.tensor.matmul(
                        S[:, b, sl, :],
                        qkT[r : r + 32, jq, e, :],
                        qkT[r : r + 32, jk, e, :],
                        start=True,
                        stop=True,
                        tile_position=(r, 0),
                    )

                # P16/Pm/Pn packed [128, 12, 128] in pidx order (ee-major);
                # full softmax chain split per ee for latency overlap
                P16 = sbp.tile([128, 12, 128], bf16, tag="P16")
                Pm = sbp.tile([128, 12, 128], bf16, tag="Pm")
                rsum = sbp.tile([128, 12], fp32, tag="rsum")
                rrec2 = sbp.tile([128, 12, 2], bf16, tag="rrec2")
                Pn = sbp.tile([128, 12, 128], bf16, tag="Pn")
                PT = sbp.tile([128, 6, 2, 128], bf16, tag="PT")
                PTp = pt("vp2", [128, 6, 2, 128], bf16, name=f"PTp_{half}")
                for ee in range(2):
                    lo = 6 * ee
                    nc.scalar.activation(
                        P16[:, lo : lo + 6, :].rearrange(
                            "p (a b) s -> p a b s", a=3
                        ),
                        S[:, :, 2 * ee : 2 * ee + 2, :],
                        AF.Exp,
                    )
                    nc.vector.tensor_mul(
                        Pm[:, lo : lo + 6, :],
                        P16[:, lo : lo + 6, :],
                        trilR[:, 0:6, :],
                    )
                    nc.vector.reduce_sum(
                        rsum[:, lo : lo + 6],
                        Pm[:, lo : lo + 6, :],
                        axis=mybir.AxisListType.X,
                    )
                    with nc.allow_low_precision(
                        "bf16 softmax reciprocal fine at 2e-2 tolerance"
                    ):
                        nc.vector.reciprocal(
                            rrec2[:, lo : lo + 6, :],
                            rsum[:, lo : lo + 6, None].broadcast_to(
                                [128, 6, 2]
                            ),
                        )
                    nc.vector.tensor_mul(
                        Pn[:, lo : lo + 6, :].rearrange(
                            "p a (c d) -> p a c d", d=2
                        ),
                        Pm[:, lo : lo + 6, :].rearrange(
                            "p a (c d) -> p a c d", d=2
                        ),
                        rrec2[:, lo : lo + 6, None, :].broadcast_to(
                            [128, 6, 64, 2]
                        ),
                    )
                    for h in range(NH):
                        _, _, pidx = SMAP[(ee, h)]
                        nc.tensor.transpose(
                            PTp[:, h, ee, :], Pn[:, pidx, :], ident[:]
                        )
                    nc.scalar.copy(PT[:, :, ee, :], PTp[:, :, ee, :])

                yt = pt("yo1", [128, 2, 2, 128], name=f"yt_{half}")
                for ee in range(2):
                    e = e0 + ee
                    for h in range(NH):
                        r = (h % 4) * 32
                        j = 0 if h < 4 else 1
                        nc.tensor.matmul(
                            yt[r : r + 32, ee, j, :],
                            v16[:, e, h * 32 : h * 32 + 32],
                            PT[:, h, ee, :],
                            start=True,
                            stop=True,
                            tile_position=(0, r),
                        )
                yT = sbp.tile([128, 2, 2, 128], bf16, tag="yT")
                nc.vector.tensor_copy(yT[:, :, 0, :], yt[:, :, 0, :])
                nc.vector.tensor_copy(yT[0:64, :, 1, :], yt[0:64, :, 1, :])
                nc.gpsimd.memset(yT[64:65, :, 1, :], 1.0)

                outs = sbp.tile([128, 2, 192], fp32, tag="outs")
                outp = pt("yo1", [128, 2, 256], name=f"outp_{half}")
                for ee in range(2):
                    nc.tensor.matmul(
                        outp[:, ee, 0:192],
                        yT[:, ee, 0, :],
                        wpA[:],
                        start=True,
                        stop=False,
                    )
                    nc.tensor.matmul(
                        outp[:, ee, 0:192],
                        yT[0:65, ee, 1, :],
                        wpB[:],
                        start=False,
                        stop=True,
                    )
                    nc.scalar.copy(outs[:, ee, :], outp[:, ee, 0:192])
                nc.sync.dma_start(
                    out_d[4 * q + e0 : 4 * q + e0 + 2].rearrange(
                        "e t c -> t e c"
                    ),
                    outs[:],
                )

    nc.finalize()
    return nc


def _prep_inputs(x, w_qkv, b_qkv, w_proj, b_proj, bl):
    bf = ml_dtypes.bfloat16
    scale = 1.0 / np.sqrt(HD)
    w2 = np.array(w_qkv, dtype=np.float32, copy=True)
    b2 = np.array(b_qkv, dtype=np.float32, copy=True)
    w2[:, 0:C] *= scale
    b2[0:C] *= scale
    # column order: [q h0-2 +pad | q h3-5 +pad | k h0-2 +pad | k h3-5 +pad | v]
    # (pad cols produce junk in unread partitions, keeping M=128 full-mode)
    perm = np.concatenate(
        [
            np.arange(0, 96),
            np.arange(0, 32),
            np.arange(96, 192),
            np.arange(0, 32),
            np.arange(192, 288),
            np.arange(0, 32),
            np.arange(288, 384),
            np.arange(0, 32),
            np.arange(384, 576),
        ]
    )
    wA = w2[0:128][:, perm].astype(bf)
    wB = np.concatenate([w2[128:192], b2[None, :]], axis=0)[:, perm].astype(bf)
    wpA = np.asarray(w_proj)[0:128].astype(bf)
    wpB = np.concatenate(
        [np.asarray(w_proj)[128:192], np.asarray(b_proj)[None, :]], axis=0
    ).astype(bf)
    trilR = np.ascontiguousarray(
        np.broadcast_to(
            np.tril(np.ones((128, 128), np.float32)), (12, 128, 128)
        ).transpose(1, 0, 2)
    ).astype(bf)
    identR = np.eye(128, dtype=np.float32).astype(bf)
    identF = np.eye(128, dtype=np.float32)
    xs = np.ascontiguousarray(np.asarray(x, dtype=np.float32)).reshape(
        -1, bl, T, C
    )
    maps = []
    for i in range(xs.shape[0]):
        maps.append(
            {
                "x": xs[i],
                "wA": wA,
                "wB": wB,
                "wpA": wpA,
                "wpB": wpB,
                "trilR": trilR,
                "identR": identR,
                "identF": identF,
            }
        )
    return maps


def _run(x, w_qkv, b_qkv, w_proj, b_proj, bl=BL, n_cores=N_CORES, trace=False):
    from concourse.bass_utils import run_bass_kernel_spmd

    key = bl
    if key not in _CACHE:
        _CACHE[key] = _build(bl)
    nc = _CACHE[key]
    maps = _prep_inputs(x, w_qkv, b_qkv, w_proj, b_proj, bl)[:n_cores]
    res = run_bass_kernel_spmd(
        nc, maps, core_ids=list(range(len(maps))), trace=trace
    )
    out = np.concatenate([r["out"] for r in res.results], axis=0)
    return out, res


def kernel(x, w_qkv, b_qkv, w_proj, b_proj):
    out, _ = _run(x, w_qkv, b_qkv, w_proj, b_proj)
    return out.reshape(B, T, C).astype(np.float32)
